# revision 17
# baseline (speedup 1.0000x reference)
"""Trainium2 Bass kernel for the ConcreteLayer training forward pass.

Computes out = x @ softmax((weight - ln(-ln((1-tiny)*uniform + tiny))) / T, axis=1)

Strategy (8 NeuronCores, 4 batch groups x 2 column halves, core = 2p+q):
  - Softmax is row-split within each HBM pair: core (p,q) computes the
    full 1024-col softmax for weight rows [q*2048, (q+1)*2048) -- row
    sums stay local, no partial-sum exchange at all.
  - The host permutes each core's weight/uniform COLUMNS so that its own
    half is always columns 0-511 (column order inside a softmax row is
    irrelevant), and permutes xt ROWS so its own rows are always k-tiles
    0-15.  This keeps the SPMD program fully static.
  - Each core keeps its own-half samples in SBUF and writes the sibling
    half to a pair-shared HBM buffer (addr_space='Shared' aliases between
    cores 2k/2k+1).  Two tiny AllGather barriers prove the writes have
    landed; they overlap with ~50us of local work (softmax + own-half
    GEMM), which also absorbs the multi-core launch skew.
  - GEMM: out[p-batch-slice, q-half] accumulates 32 k-tiles in PSUM:
    k 0-15 from local samples while the softmax streams, k 16-31 from
    the sibling's shared buffer after each barrier.  x is bf16 on host.
"""

import sys

import numpy as np

for _p in ("/opt/trn_rl_repo",):
    if _p not in sys.path:
        sys.path.insert(0, _p)

B, IN, OUT = 4096, 4096, 1024
NCORES = 8
GB, GO = 4, 2  # batch groups x column halves
BS = B // GB  # 1024 batch rows per core
OH = OUT // GO  # 512 out cols per core
RS = IN // GO  # 2048 softmax rows per core
P = 128
NST = 4  # softmax stages per core
SR = RS // NST  # 512 rows per stage
SKT = SR // P  # 4 k-tiles per stage
HKT = RS // P  # 16 own k-tiles (= sibling k-tiles)
KT = IN // P  # 32 contraction tiles
MBT = BS // P  # 8 output row tiles per core
TINY = float(np.finfo(np.float32).tiny)

_PROGRAM = None
LAST_RESULT = None


def _pin_act_tables():
    """Steer the act-table-load pass to one set (has both Ln and Exp) so the
    compiler emits one ACT_TABLE_LOAD instead of reloading per tile."""
    import concourse.mybir as mybir
    from concourse import bacc, hw_specs

    orig = hw_specs.get_activation_tables.__wrapped__
    target = "natural_log_exp_and_others"
    strip = {
        mybir.ActivationFunctionType.Ln,
        mybir.ActivationFunctionType.Exp,
    }

    def pinned(arch):
        tables = orig(arch)
        if target not in tables:
            return tables
        return {
            name: (set(fns) if name == target else {f for f in fns if f not in strip})
            for name, fns in tables.items()
        }

    bacc.get_activation_tables = pinned


def _build_program():
    import concourse.bass as bass
    import concourse.mybir as mybir
    import concourse.tile as tile
    from concourse import bacc
    from contextlib import ExitStack

    _pin_act_tables()

    f32 = mybir.dt.float32
    bf16 = mybir.dt.bfloat16
    u8 = mybir.dt.uint8
    Ln = mybir.ActivationFunctionType.Ln
    Exp = mybir.ActivationFunctionType.Exp

    nc = bacc.Bacc(
        "TRN2", target_bir_lowering=False, debug=False, num_devices=NCORES
    )

    xt_d = nc.dram_tensor("xt", [IN, BS], bf16, kind="ExternalInput")
    wh_d = nc.dram_tensor("wh", [RS, OUT], f32, kind="ExternalInput")
    uh_d = nc.dram_tensor("uh", [RS, OUT], f32, kind="ExternalInput")
    t_d = nc.dram_tensor("tt", [1], f32, kind="ExternalInput")
    out_d = nc.dram_tensor("out", [BS, OH], f32, kind="ExternalOutput")

    replica_groups = [[0, 1, 2, 3, 4, 5, 6, 7]]

    with tile.TileContext(nc) as tc, ExitStack() as ctx:
        dram = ctx.enter_context(tc.tile_pool(name="dram", bufs=1, space="DRAM"))
        singles = ctx.enter_context(tc.tile_pool(name="singles", bufs=1))
        chunks = ctx.enter_context(tc.tile_pool(name="chunks", bufs=2))
        outp = ctx.enter_context(tc.tile_pool(name="outp", bufs=2))
        psum = ctx.enter_context(tc.tile_pool(name="psum", bufs=1, space="PSUM"))

        # Pair-shared sample exchange buffers (one per stage -- a Shared
        # tile may only have a single writer instruction): slot [parity]
        # holds that writer's sibling-half samples.
        shr = [
            dram.tile([GO, P, SKT, OH], bf16, name=f"pairx{s}", addr_space="Shared")
            for s in range(NST)
        ]
        # Barrier collectives (tiny).
        bar_in = [dram.tile([1, 4], f32, name=f"bar_in{a}") for a in range(2)]
        bar_out = [
            dram.tile([NCORES, 4], f32, name=f"bar_out{a}", addr_space="Shared")
            for a in range(2)
        ]

        # 1/T broadcast to all partitions.
        t_sb = singles.tile([P, 1], f32)
        t_ap = t_d.ap()
        nc.sync.dma_start(
            out=t_sb, in_=bass.AP(tensor=t_ap.tensor, offset=0, ap=[[0, P], [1, 1]])
        )
        invt = singles.tile([P, 1], f32)
        nc.vector.reciprocal(invt, t_sb)

        zero_t = singles.tile([P, 1], f32)
        nc.vector.memset(zero_t, 0.0)
        tiny_t = singles.tile([P, 1], f32)
        nc.vector.memset(tiny_t, TINY)

        parity = nc.sync.partition_id() % 2

        # u/w stage loads, all issued up front (pool-cycled, bufs=2).
        u_ts, w_ts = [], []
        for s in range(NST):
            u_t = chunks.tile([P, SKT, OUT], f32, tag="u", name="u_t")
            w_t = chunks.tile([P, SKT, OUT], f32, tag="w", name="w_t")
            nc.sync.dma_start(
                out=u_t,
                in_=uh_d[s * SR : (s + 1) * SR, :].rearrange("(k p) c -> p k c", p=P),
            )
            nc.sync.dma_start(
                out=w_t,
                in_=wh_d[s * SR : (s + 1) * SR, :].rearrange("(k p) c -> p k c", p=P),
            )
            u_ts.append(u_t)
            w_ts.append(w_t)

        # Resident xT (bf16, host-permuted: own rows first), k-tile major.
        xt_all = singles.tile([P, KT, BS], bf16)
        xt_v = xt_d[:].rearrange("(g p) b -> p g b", p=P)
        for qtr in range(4):
            nc.gpsimd.dma_start(
                out=xt_all[:, qtr * 8 : (qtr + 1) * 8, :],
                in_=xt_v[:, qtr * 8 : (qtr + 1) * 8, :],
            )

        # Own-half samples (k-tiles 0-15) and sibling-half staging.
        e_own = singles.tile([P, HKT, OH], bf16)
        e_sib = singles.tile([P, HKT, OH], bf16)

        def softmax_stage(s):
            u_t, w_t = u_ts[s], w_ts[s]
            # v = ln((1 - tiny)*u + tiny); m = ln(-v) = -gumbel
            nc.scalar.activation(u_t, u_t, Ln, bias=tiny_t[:], scale=1.0 - TINY)
            nc.scalar.activation(u_t, u_t, Ln, bias=zero_t[:], scale=-1.0)
            # d = w - m = w + gumbel
            nc.vector.tensor_sub(u_t, w_t, u_t)
            esh = chunks.tile([P, SKT, OH], bf16, tag="esh", name="esh")
            for kt in range(SKT):
                g = s * SKT + kt
                ef = chunks.tile([P, OUT], bf16, tag="ef", name="ef")
                sums = chunks.tile([P, 1], f32, tag="sums", name="sums")
                nc.scalar.activation(
                    ef, u_t[:, kt, :], Exp, bias=zero_t[:], scale=invt[:],
                    accum_out=sums,
                )
                rsum = chunks.tile([P, 1], f32, tag="rsum", name="rsum")
                nc.vector.reciprocal(rsum, sums)
                # own half (host-permuted to cols 0-511) -> e_own; sibling
                # half -> staging for the pair-shared write.
                nc.vector.tensor_scalar_mul(e_own[:, g, :], ef[:, :OH], rsum)
                nc.vector.tensor_scalar_mul(esh[:, kt, :], ef[:, OH:], rsum)
            return esh

        esh_ts = [softmax_stage(s) for s in range(NST)]
        # Sibling-half stages into my pair-shared slot (after all wu loads
        # in the sync stream).
        for s in range(NST):
            nc.sync.dma_start(out=shr[s][bass.ts(parity, 1), :, :, :], in_=esh_ts[s])

        # Barriers: a in {0,1} covers stages [2a, 2a+1].  The read-back of
        # my own slot (RAW on the shared writes) orders the collective
        # trigger after the writes have fully landed in HBM.
        brd = []
        for a in range(2):
            rb0 = singles.tile([P, 8], bf16, name=f"brda{a}")
            nc.sync.dma_start(
                out=rb0, in_=shr[2 * a][bass.ts(parity, 1), :, SKT - 1, OH - 8 : OH]
            )
            rb1 = singles.tile([P, 8], bf16, name=f"brdb{a}")
            nc.sync.dma_start(
                out=rb1,
                in_=shr[2 * a + 1][bass.ts(parity, 1), :, SKT - 1, OH - 8 : OH],
            )
            brd.append((rb0, rb1))
        bar_sb = []
        for a in range(2):
            bin_sb = singles.tile([1, 4], f32, name=f"bin{a}")
            # derive the barrier payload from both readbacks so the
            # collective input depends on both shared-write completions
            nc.vector.tensor_copy(bin_sb[:, 0:2], brd[a][0][0:1, 0:2])
            nc.vector.tensor_copy(bin_sb[:, 2:4], brd[a][1][0:1, 0:2])
            nc.sync.dma_start(out=bar_in[a], in_=bin_sb)
            nc.gpsimd.collective_compute(
                "AllGather",
                mybir.AluOpType.bypass,
                replica_groups=replica_groups,
                ins=[bar_in[a].opt()],
                outs=[bar_out[a].opt()],
            )
            bout = singles.tile([NCORES, 4], f32, name=f"bout{a}")
            nc.sync.dma_start(out=bout, in_=bar_out[a][:])
            bar_sb.append(bout)

        # Sibling sample read-back, gated on the matching barrier via a
        # tiny DVE pre-write of the destination tile (WAW for the DMA).
        for a in range(2):
            for j in range(2):
                st = 2 * a + j
                nc.vector.tensor_copy(e_sib[0:1, st * SKT, 0:4], bar_sb[a][0:1, :])
                nc.sync.dma_start(
                    out=e_sib[:, st * SKT : (st + 1) * SKT, :],
                    in_=shr[st][bass.ts(1 - parity, 1), :, :, :].rearrange(
                        "o p k c -> p o k c"
                    ),
                )

        ps_tiles = [
            psum.tile([P, OH], f32, tag=f"ps{mb}", name=f"ps{mb}")
            for mb in range(MBT)
        ]

        def gemm_k(g, e_src, gl, start, stop):
            for mb in range(MBT):
                nc.tensor.matmul(
                    ps_tiles[mb][:],
                    lhsT=xt_all[:, g, mb * P : (mb + 1) * P],
                    rhs=e_src[:, gl, :],
                    start=start,
                    stop=stop,
                )

        # k 0-15: own samples while softmax streams; k 16-31: sibling.
        for g in range(HKT):
            gemm_k(g, e_own, g, start=(g == 0), stop=False)
        for g in range(HKT):
            gemm_k(HKT + g, e_sib, g, start=False, stop=(g == HKT - 1))

        for mb in range(MBT):
            o_t = outp.tile([P, OH], f32, tag="o")
            nc.vector.tensor_copy(o_t, ps_tiles[mb][:])
            nc.sync.dma_start(out=out_d[mb * P : (mb + 1) * P, :], in_=o_t)

    nc.compile()
    return nc


def kernel(x, weight, uniform, T):
    global _PROGRAM, LAST_RESULT
    import ml_dtypes
    from concourse.bass_utils import run_bass_kernel_spmd

    if _PROGRAM is None:
        _PROGRAM = _build_program()
    nc = _PROGRAM

    x = np.asarray(x, dtype=np.float32)
    weight = np.asarray(weight, dtype=np.float32)
    uniform = np.asarray(uniform, dtype=np.float32)
    T = np.ascontiguousarray(np.asarray(T, dtype=np.float32)).reshape([1])

    xt = np.ascontiguousarray(x.T).astype(ml_dtypes.bfloat16)  # [IN, B] bf16
    in_maps = []
    for c in range(NCORES):
        p, q = c // GO, c % GO
        rows_own = slice(q * RS, (q + 1) * RS)
        rows_sib = slice((1 - q) * RS, (2 - q) * RS)
        cols_perm = np.r_[q * OH : (q + 1) * OH, (1 - q) * OH : (2 - q) * OH]
        xt_perm = np.concatenate([xt[rows_own], xt[rows_sib]], axis=0)
        in_maps.append(
            {
                "xt": np.ascontiguousarray(xt_perm[:, p * BS : (p + 1) * BS]),
                "wh": np.ascontiguousarray(weight[rows_own][:, cols_perm]),
                "uh": np.ascontiguousarray(uniform[rows_own][:, cols_perm]),
                "tt": T,
            }
        )

    res = run_bass_kernel_spmd(nc, in_maps, core_ids=list(range(NCORES)))
    LAST_RESULT = res

    out = np.empty((B, OUT), dtype=np.float32)
    for c in range(NCORES):
        p, q = c // GO, c % GO
        out[p * BS : (p + 1) * BS, q * OH : (q + 1) * OH] = res.results[c]["out"]
    return out


# revision 20
# speedup vs baseline: 1.0893x; 1.0893x over previous
"""Trainium2 Bass kernel for the ConcreteLayer training forward pass.

Computes out = x @ softmax((weight - ln(-ln((1-tiny)*uniform + tiny))) / T, axis=1)

Strategy (8 NeuronCores, 4 batch groups x 2 column halves, core = 2p+q):
  - Softmax is row-split within each HBM pair: core (p,q) computes the
    full 1024-col softmax for weight rows [q*2048, (q+1)*2048) -- row
    sums stay local, no partial-sum exchange at all.
  - The host permutes each core's weight/uniform COLUMNS so that its own
    half is always columns 0-511 (column order inside a softmax row is
    irrelevant), and permutes xt ROWS so its own rows are always k-tiles
    0-15.  This keeps the SPMD program fully static.
  - Each core keeps its own-half samples in SBUF and writes the sibling
    half to a pair-shared HBM buffer (addr_space='Shared' aliases between
    cores 2k/2k+1).  A tiny dummy AllGather aligns the cores early
    (absorbing multi-core launch skew under local work); four per-stage
    AllGather barriers then prove the shared writes have landed.
  - GEMM: out[p-batch-slice, q-half] accumulates 32 k-tiles in PSUM:
    k 0-15 from local samples while the softmax streams, k 16-31 from
    the sibling's shared buffer after each barrier.  x is bf16 on host.
"""

import sys

import numpy as np

for _p in ("/opt/trn_rl_repo",):
    if _p not in sys.path:
        sys.path.insert(0, _p)

B, IN, OUT = 4096, 4096, 1024
NCORES = 8
GB, GO = 4, 2  # batch groups x column halves
BS = B // GB  # 1024 batch rows per core
OH = OUT // GO  # 512 out cols per core
RS = IN // GO  # 2048 softmax rows per core
P = 128
NST = 4  # softmax stages per core (barrier granularity)
SKT = 4  # k-tiles per stage
HKT = NST * SKT  # 16 own k-tiles (= sibling k-tiles)
KT = IN // P  # 32 contraction tiles
MBT = BS // P  # 8 output row tiles per core
TINY = float(np.finfo(np.float32).tiny)

_PROGRAM = None
LAST_RESULT = None


def _pin_act_tables():
    """Steer the act-table-load pass to one set (has both Ln and Exp) so the
    compiler emits one ACT_TABLE_LOAD instead of reloading per tile."""
    import concourse.mybir as mybir
    from concourse import bacc, hw_specs

    orig = hw_specs.get_activation_tables.__wrapped__
    target = "natural_log_exp_and_others"
    strip = {
        mybir.ActivationFunctionType.Ln,
        mybir.ActivationFunctionType.Exp,
    }

    def pinned(arch):
        tables = orig(arch)
        if target not in tables:
            return tables
        return {
            name: (set(fns) if name == target else {f for f in fns if f not in strip})
            for name, fns in tables.items()
        }

    bacc.get_activation_tables = pinned


def _build_program():
    import concourse.bass as bass
    import concourse.mybir as mybir
    import concourse.tile as tile
    from concourse import bacc
    from contextlib import ExitStack

    _pin_act_tables()

    f32 = mybir.dt.float32
    bf16 = mybir.dt.bfloat16
    Ln = mybir.ActivationFunctionType.Ln
    Exp = mybir.ActivationFunctionType.Exp

    nc = bacc.Bacc(
        "TRN2", target_bir_lowering=False, debug=False, num_devices=NCORES
    )

    xt_d = nc.dram_tensor("xt", [IN, BS], bf16, kind="ExternalInput")
    wh_d = nc.dram_tensor("wh", [RS, OUT], f32, kind="ExternalInput")
    uh_d = nc.dram_tensor("uh", [RS, OUT], f32, kind="ExternalInput")
    t_d = nc.dram_tensor("tt", [1], f32, kind="ExternalInput")
    out_d = nc.dram_tensor("out", [BS, OH], f32, kind="ExternalOutput")

    replica_groups = [[0, 1, 2, 3, 4, 5, 6, 7]]

    with tile.TileContext(nc) as tc, ExitStack() as ctx:
        dram = ctx.enter_context(tc.tile_pool(name="dram", bufs=1, space="DRAM"))
        singles = ctx.enter_context(tc.tile_pool(name="singles", bufs=1))
        chunks = ctx.enter_context(tc.tile_pool(name="chunks", bufs=6))
        eshp = ctx.enter_context(tc.tile_pool(name="eshp", bufs=2))
        outp = ctx.enter_context(tc.tile_pool(name="outp", bufs=2))
        psum = ctx.enter_context(tc.tile_pool(name="psum", bufs=1, space="PSUM"))

        # Pair-shared sample exchange buffers (one per stage -- a Shared
        # tile may only have a single writer instruction): slot [parity]
        # holds that writer's sibling-half samples.
        shr = [
            dram.tile([GO, P, SKT, OH], bf16, name=f"pairx{s}", addr_space="Shared")
            for s in range(NST)
        ]
        # Alignment + per-stage barrier collectives (tiny).
        NBAR = NST + 1  # [0] = early alignment barrier, [1+s] = stage s
        bar_in = [dram.tile([1, 4], f32, name=f"bar_in{a}") for a in range(NBAR)]
        bar_out = [
            dram.tile([NCORES, 4], f32, name=f"bar_out{a}", addr_space="Shared")
            for a in range(NBAR)
        ]

        # 1/T broadcast to all partitions.
        t_sb = singles.tile([P, 1], f32)
        t_ap = t_d.ap()
        nc.sync.dma_start(
            out=t_sb, in_=bass.AP(tensor=t_ap.tensor, offset=0, ap=[[0, P], [1, 1]])
        )
        invt = singles.tile([P, 1], f32)
        nc.vector.reciprocal(invt, t_sb)

        zero_t = singles.tile([P, 1], f32)
        nc.vector.memset(zero_t, 0.0)
        tiny_t = singles.tile([P, 1], f32)
        nc.vector.memset(tiny_t, TINY)

        parity = nc.sync.partition_id() % 2
        parity_gp = nc.gpsimd.partition_id() % 2

        # u/w loads per k-tile (128 rows), all issued up front in k order;
        # the 6-deep pool keeps the DMA comfortably ahead of the ACT chain
        # without head-of-line blocking the sync queue.
        u_ts, w_ts = [], []
        for g in range(HKT):
            u_t = chunks.tile([P, OUT], f32, tag="u", name="u_t")
            w_t = chunks.tile([P, OUT], f32, tag="w", name="w_t")
            nc.sync.dma_start(out=u_t, in_=uh_d[g * P : (g + 1) * P, :])
            nc.sync.dma_start(out=w_t, in_=wh_d[g * P : (g + 1) * P, :])
            u_ts.append(u_t)
            w_ts.append(w_t)

        # Resident xT (bf16, host-permuted: own rows first), k-tile major.
        xt_all = singles.tile([P, KT, BS], bf16)
        xt_v = xt_d[:].rearrange("(g p) b -> p g b", p=P)
        for qtr in range(4):
            nc.gpsimd.dma_start(
                out=xt_all[:, qtr * 8 : (qtr + 1) * 8, :],
                in_=xt_v[:, qtr * 8 : (qtr + 1) * 8, :],
            )

        # Early alignment barrier: no data deps, fires as soon as the
        # gpsimd stream reaches it; all cores leave it nearly in lockstep.
        bind = singles.tile([1, 4], f32, name="bind")
        nc.vector.memset(bind, 1.0)
        nc.sync.dma_start(out=bar_in[0], in_=bind)
        nc.gpsimd.collective_compute(
            "AllGather",
            mybir.AluOpType.bypass,
            replica_groups=replica_groups,
            ins=[bar_in[0].opt()],
            outs=[bar_out[0].opt()],
        )

        # Own-half samples (k-tiles 0-15) and sibling-half destination.
        e_own = singles.tile([P, HKT, OH], bf16)
        e_sib = singles.tile([P, HKT, OH], bf16)

        def softmax_ktile(g, esh):
            u_t, w_t = u_ts[g], w_ts[g]
            # v = ln((1 - tiny)*u + tiny); m = ln(-v) = -gumbel
            nc.scalar.activation(u_t, u_t, Ln, bias=tiny_t[:], scale=1.0 - TINY)
            nc.scalar.activation(u_t, u_t, Ln, bias=zero_t[:], scale=-1.0)
            # d = w - m = w + gumbel
            nc.vector.tensor_sub(u_t, w_t, u_t)
            ef = chunks.tile([P, OUT], bf16, tag="ef", name="ef", bufs=2)
            sums = chunks.tile([P, 1], f32, tag="sums", name="sums", bufs=2)
            nc.scalar.activation(
                ef, u_t, Exp, bias=zero_t[:], scale=invt[:], accum_out=sums
            )
            rsum = chunks.tile([P, 1], f32, tag="rsum", name="rsum", bufs=2)
            nc.vector.reciprocal(rsum, sums)
            # own half (host-permuted to cols 0-511) -> e_own; sibling
            # half -> staging tile for the pair-shared write.
            nc.vector.tensor_scalar_mul(e_own[:, g, :], ef[:, :OH], rsum)
            nc.vector.tensor_scalar_mul(esh[:, g % SKT, :], ef[:, OH:], rsum)

        esh_ts = []
        for s in range(NST):
            esh = eshp.tile([P, SKT, OH], bf16, tag="esh", name="esh")
            for kt in range(SKT):
                softmax_ktile(s * SKT + kt, esh)
            esh_ts.append(esh)

        # Per-stage: sibling-half write to the pair-shared slot, then a
        # small readback of the written region (RAW -> completion proof),
        # then the barrier input derived from the readback.
        for s in range(NST):
            nc.sync.dma_start(
                out=shr[s][bass.ts(parity, 1), :, :, :], in_=esh_ts[s]
            )
            brd = singles.tile([P, 8], bf16, name=f"brd{s}")
            nc.sync.dma_start(
                out=brd, in_=shr[s][bass.ts(parity, 1), :, SKT - 1, OH - 8 : OH]
            )
            bin_sb = singles.tile([1, 4], f32, name=f"bin{s}")
            nc.vector.tensor_copy(bin_sb, brd[0:1, 0:4])
            nc.sync.dma_start(out=bar_in[1 + s], in_=bin_sb)

        # Stage barriers + sibling readbacks on the gpsimd queue.
        bar_sb = []
        for s in range(NST):
            nc.gpsimd.collective_compute(
                "AllGather",
                mybir.AluOpType.bypass,
                replica_groups=replica_groups,
                ins=[bar_in[1 + s].opt()],
                outs=[bar_out[1 + s].opt()],
            )
            bout = singles.tile([NCORES, 4], f32, name=f"bout{s}")
            nc.gpsimd.dma_start(out=bout, in_=bar_out[1 + s][:])
            bar_sb.append(bout)
            # barrier-gate the readback via a DVE pre-write (WAW for DMA)
            nc.vector.tensor_copy(e_sib[0:1, s * SKT, 0:4], bout[0:1, :])
            nc.gpsimd.dma_start(
                out=e_sib[:, s * SKT : (s + 1) * SKT, :],
                in_=shr[s][bass.ts(1 - parity_gp, 1), :, :, :].rearrange(
                    "o p k c -> p o k c"
                ),
            )

        ps_tiles = [
            psum.tile([P, OH], f32, tag=f"ps{mb}", name=f"ps{mb}")
            for mb in range(MBT)
        ]

        def gemm_k(g, e_src, gl, start, stop):
            for mb in range(MBT):
                nc.tensor.matmul(
                    ps_tiles[mb][:],
                    lhsT=xt_all[:, g, mb * P : (mb + 1) * P],
                    rhs=e_src[:, gl, :],
                    start=start,
                    stop=stop,
                )

        # k 0-15: own samples while softmax streams; k 16-31: sibling.
        for g in range(HKT):
            gemm_k(g, e_own, g, start=(g == 0), stop=False)
        for g in range(HKT):
            gemm_k(HKT + g, e_sib, g, start=False, stop=(g == HKT - 1))

        for mb in range(MBT):
            o_t = outp.tile([P, OH], f32, tag="o")
            nc.vector.tensor_copy(o_t, ps_tiles[mb][:])
            nc.sync.dma_start(out=out_d[mb * P : (mb + 1) * P, :], in_=o_t)

    nc.compile()
    return nc


def kernel(x, weight, uniform, T):
    global _PROGRAM, LAST_RESULT
    import ml_dtypes
    from concourse.bass_utils import run_bass_kernel_spmd

    if _PROGRAM is None:
        _PROGRAM = _build_program()
    nc = _PROGRAM

    x = np.asarray(x, dtype=np.float32)
    weight = np.asarray(weight, dtype=np.float32)
    uniform = np.asarray(uniform, dtype=np.float32)
    T = np.ascontiguousarray(np.asarray(T, dtype=np.float32)).reshape([1])

    xt = np.ascontiguousarray(x.T).astype(ml_dtypes.bfloat16)  # [IN, B] bf16
    in_maps = []
    for c in range(NCORES):
        p, q = c // GO, c % GO
        rows_own = slice(q * RS, (q + 1) * RS)
        rows_sib = slice((1 - q) * RS, (2 - q) * RS)
        cols_perm = np.r_[q * OH : (q + 1) * OH, (1 - q) * OH : (2 - q) * OH]
        xt_perm = np.concatenate([xt[rows_own], xt[rows_sib]], axis=0)
        in_maps.append(
            {
                "xt": np.ascontiguousarray(xt_perm[:, p * BS : (p + 1) * BS]),
                "wh": np.ascontiguousarray(weight[rows_own][:, cols_perm]),
                "uh": np.ascontiguousarray(uniform[rows_own][:, cols_perm]),
                "tt": T,
            }
        )

    res = run_bass_kernel_spmd(nc, in_maps, core_ids=list(range(NCORES)))
    LAST_RESULT = res

    out = np.empty((B, OUT), dtype=np.float32)
    for c in range(NCORES):
        p, q = c // GO, c % GO
        out[p * BS : (p + 1) * BS, q * OH : (q + 1) * OH] = res.results[c]["out"]
    return out


# revision 26
# speedup vs baseline: 1.2698x; 1.1656x over previous
"""Trainium2 Bass kernel for the ConcreteLayer training forward pass.

Computes out = x @ softmax((weight - ln(-ln((1-tiny)*uniform + tiny))) / T, axis=1)

Strategy (8 NeuronCores, 4 batch groups x 2 column halves, core = 2p+q):
  - Softmax is row-split within each HBM pair: core (p,q) computes the
    full 1024-col softmax for weight rows [q*2048, (q+1)*2048) -- row
    sums stay local, no partial-sum exchange at all.
  - The host permutes each core's weight/uniform COLUMNS so that its own
    half is always columns 0-511 (column order inside a softmax row is
    irrelevant), and permutes xt ROWS so its own rows are always k-tiles
    0-15.  This keeps the SPMD program fully static.
  - Each core keeps its own-half samples in SBUF and writes the sibling
    half to a pair-shared HBM buffer (addr_space='Shared' aliases between
    cores 2k/2k+1).  A tiny dummy AllGather aligns the cores early
    (absorbing multi-core launch skew under local work); four per-stage
    AllGather barriers then prove the shared writes have landed.
  - GEMM: out[p-batch-slice, q-half] accumulates 32 k-tiles in PSUM:
    k 0-15 from local samples while the softmax streams, k 16-31 from
    the sibling's shared buffer after each barrier.  x is bf16 on host.
"""

import sys

import numpy as np

for _p in ("/opt/trn_rl_repo",):
    if _p not in sys.path:
        sys.path.insert(0, _p)

B, IN, OUT = 4096, 4096, 1024
NCORES = 8
GB, GO = 4, 2  # batch groups x column halves
BS = B // GB  # 1024 batch rows per core
OH = OUT // GO  # 512 out cols per core
RS = IN // GO  # 2048 softmax rows per core
P = 128
NST = 4  # softmax stages per core (barrier granularity)
SKT = 4  # k-tiles per stage
HKT = NST * SKT  # 16 own k-tiles (= sibling k-tiles)
KT = IN // P  # 32 contraction tiles
MBT = BS // P  # 8 output row tiles per core
TINY = float(np.finfo(np.float32).tiny)

_PROGRAM = None
LAST_RESULT = None


def _pin_act_tables():
    """Steer the act-table-load pass to one set (has both Ln and Exp) so the
    compiler emits one ACT_TABLE_LOAD instead of reloading per tile."""
    import concourse.mybir as mybir
    from concourse import bacc, hw_specs

    orig = hw_specs.get_activation_tables.__wrapped__
    target = "natural_log_exp_and_others"
    strip = {
        mybir.ActivationFunctionType.Ln,
        mybir.ActivationFunctionType.Exp,
    }

    def pinned(arch):
        tables = orig(arch)
        if target not in tables:
            return tables
        return {
            name: (set(fns) if name == target else {f for f in fns if f not in strip})
            for name, fns in tables.items()
        }

    bacc.get_activation_tables = pinned


def _build_program():
    import concourse.bass as bass
    import concourse.mybir as mybir
    import concourse.tile as tile
    from concourse import bacc
    from contextlib import ExitStack

    _pin_act_tables()

    f32 = mybir.dt.float32
    bf16 = mybir.dt.bfloat16
    Ln = mybir.ActivationFunctionType.Ln
    Exp = mybir.ActivationFunctionType.Exp

    nc = bacc.Bacc(
        "TRN2", target_bir_lowering=False, debug=False, num_devices=NCORES
    )

    # host-pretiled: xt_d[p, g*BS + b] = xT[g*128 + p, b] (own rows first)
    xt_d = nc.dram_tensor("xt", [P, KT * BS], bf16, kind="ExternalInput")
    wh_d = nc.dram_tensor("wh", [RS, OUT], f32, kind="ExternalInput")
    uh_d = nc.dram_tensor("uh", [RS, OUT], f32, kind="ExternalInput")
    t_d = nc.dram_tensor("tt", [1], f32, kind="ExternalInput")
    out_d = nc.dram_tensor("out", [BS, OH], f32, kind="ExternalOutput")

    replica_groups = [[0, 1, 2, 3, 4, 5, 6, 7]]

    with tile.TileContext(nc) as tc, ExitStack() as ctx:
        dram = ctx.enter_context(tc.tile_pool(name="dram", bufs=1, space="DRAM"))
        singles = ctx.enter_context(tc.tile_pool(name="singles", bufs=1))
        chunks = ctx.enter_context(tc.tile_pool(name="chunks", bufs=6))
        eshp = ctx.enter_context(tc.tile_pool(name="eshp", bufs=2))
        outp = ctx.enter_context(tc.tile_pool(name="outp", bufs=2))
        psum = ctx.enter_context(tc.tile_pool(name="psum", bufs=1, space="PSUM"))

        # Pair-shared sample exchange buffers (one per stage -- a Shared
        # tile may only have a single writer instruction): slot [parity]
        # holds that writer's sibling-half samples.
        shr = [
            dram.tile([GO, P, SKT, OH], bf16, name=f"pairx{s}", addr_space="Shared")
            for s in range(NST)
        ]
        # Alignment + per-stage barrier collectives (tiny).
        NBAR = NST + 1  # [0] = early alignment barrier, [1+s] = stage s
        bar_in = [dram.tile([1, 4], f32, name=f"bar_in{a}") for a in range(NBAR)]
        bar_out = [
            dram.tile([NCORES, 4], f32, name=f"bar_out{a}", addr_space="Shared")
            for a in range(NBAR)
        ]

        # 1/T broadcast to all partitions.
        t_sb = singles.tile([P, 1], f32)
        t_ap = t_d.ap()
        nc.sync.dma_start(
            out=t_sb, in_=bass.AP(tensor=t_ap.tensor, offset=0, ap=[[0, P], [1, 1]])
        )
        invt = singles.tile([P, 1], f32)
        nc.vector.reciprocal(invt, t_sb)

        zero_t = singles.tile([P, 1], f32)
        nc.vector.memset(zero_t, 0.0)
        tiny_t = singles.tile([P, 1], f32)
        nc.vector.memset(tiny_t, TINY)

        parity = nc.sync.partition_id() % 2
        parity_act = nc.scalar.partition_id() % 2

        # u/w loads per k-tile (128 rows), all issued up front in k order;
        # the 6-deep pool keeps the DMA comfortably ahead of the ACT chain
        # without head-of-line blocking the sync queue.
        u_ts, w_ts = [], []
        for g in range(HKT):
            u_t = chunks.tile([P, OUT], f32, tag="u", name="u_t")
            w_t = chunks.tile([P, OUT], f32, tag="w", name="w_t")
            nc.sync.dma_start(out=u_t, in_=uh_d[g * P : (g + 1) * P, :])
            nc.sync.dma_start(out=w_t, in_=wh_d[g * P : (g + 1) * P, :])
            u_ts.append(u_t)
            w_ts.append(w_t)

        # Resident xT (bf16, host-pretiled to [P, KT*BS]), loaded in 4
        # contiguous chunks on the ACT HWDGE ring (interleaved below).
        xt_all = singles.tile([P, KT, BS], bf16)

        def load_xt_quarter(qtr):
            nc.scalar.dma_start(
                out=xt_all[:, qtr * 8 : (qtr + 1) * 8, :],
                in_=xt_d[:, qtr * 8 * BS : (qtr + 1) * 8 * BS],
            )

        # Early alignment barrier: no data deps, fires as soon as the
        # gpsimd stream reaches it; all cores leave it nearly in lockstep.
        bind = singles.tile([1, 4], f32, name="bind")
        nc.vector.memset(bind, 1.0)
        nc.sync.dma_start(out=bar_in[0], in_=bind)
        nc.gpsimd.collective_compute(
            "AllGather",
            mybir.AluOpType.bypass,
            replica_groups=replica_groups,
            ins=[bar_in[0].opt()],
            outs=[bar_out[0].opt()],
        )

        # Own-half samples (k-tiles 0-15) and sibling-half destination.
        e_own = singles.tile([P, HKT, OH], bf16)
        e_sib = singles.tile([P, HKT, OH], bf16)

        def softmax_ktile(g, esh):
            u_t, w_t = u_ts[g], w_ts[g]
            # v = ln((1 - tiny)*u + tiny); m = ln(-v) = -gumbel
            nc.scalar.activation(u_t, u_t, Ln, bias=tiny_t[:], scale=1.0 - TINY)
            nc.scalar.activation(u_t, u_t, Ln, bias=zero_t[:], scale=-1.0)
            # d = w - m = w + gumbel
            nc.vector.tensor_sub(u_t, w_t, u_t)
            ef = chunks.tile([P, OUT], bf16, tag="ef", name="ef", bufs=2)
            sums = chunks.tile([P, 1], f32, tag="sums", name="sums", bufs=2)
            nc.scalar.activation(
                ef, u_t, Exp, bias=zero_t[:], scale=invt[:], accum_out=sums
            )
            rsum = chunks.tile([P, 1], f32, tag="rsum", name="rsum", bufs=2)
            nc.vector.reciprocal(rsum, sums)
            # own half (host-permuted to cols 0-511) -> e_own; sibling
            # half -> staging tile for the pair-shared write.
            nc.vector.tensor_scalar_mul(e_own[:, g, :], ef[:, :OH], rsum)
            nc.vector.tensor_scalar_mul(esh[:, g % SKT, :], ef[:, OH:], rsum)

        def stage_barrier(s, esh):
            # sibling-half write to the pair-shared slot, then a small
            # readback of the written region (RAW -> completion proof),
            # then the barrier input derived from the readback, then the
            # barrier collective itself + its output readback.
            nc.sync.dma_start(out=shr[s][bass.ts(parity, 1), :, :, :], in_=esh)
            brd = singles.tile([P, 8], bf16, name=f"brd{s}")
            nc.sync.dma_start(
                out=brd, in_=shr[s][bass.ts(parity, 1), :, SKT - 1, OH - 8 : OH]
            )
            bin_sb = singles.tile([1, 4], f32, name=f"bin{s}")
            nc.vector.tensor_copy(bin_sb, brd[0:1, 0:4])
            nc.sync.dma_start(out=bar_in[1 + s], in_=bin_sb)
            nc.gpsimd.collective_compute(
                "AllGather",
                mybir.AluOpType.bypass,
                replica_groups=replica_groups,
                ins=[bar_in[1 + s].opt()],
                outs=[bar_out[1 + s].opt()],
            )
            bout = singles.tile([NCORES, 4], f32, name=f"bout{s}")
            nc.gpsimd.dma_start(out=bout, in_=bar_out[1 + s][:])
            return bout

        def sib_readback(s, bout):
            # barrier-gate the readback via a DVE pre-write (WAW for the
            # DMA); the readback itself rides the ACT HWDGE ring.
            nc.vector.tensor_copy(e_sib[0:1, s * SKT, 0:4], bout[0:1, :])
            nc.scalar.dma_start(
                out=e_sib[:, s * SKT : (s + 1) * SKT, :],
                in_=shr[s][bass.ts(1 - parity_act, 1), :, :, :].rearrange(
                    "o p k c -> p o k c"
                ),
            )

        bouts = []
        for s in range(NST):
            load_xt_quarter(s)
            esh = eshp.tile([P, SKT, OH], bf16, tag="esh", name="esh")
            for kt in range(SKT):
                softmax_ktile(s * SKT + kt, esh)
            bouts.append(stage_barrier(s, esh))
            if s >= 1:
                sib_readback(s - 1, bouts[s - 1])
        sib_readback(NST - 1, bouts[NST - 1])

        ps_tiles = [
            psum.tile([P, OH], f32, tag=f"ps{mb}", name=f"ps{mb}")
            for mb in range(MBT)
        ]

        def gemm_k(g, e_src, gl, start, stop):
            for mb in range(MBT):
                nc.tensor.matmul(
                    ps_tiles[mb][:],
                    lhsT=xt_all[:, g, mb * P : (mb + 1) * P],
                    rhs=e_src[:, gl, :],
                    start=start,
                    stop=stop,
                )

        # k 0-15: own samples while softmax streams; k 16-31: sibling.
        for g in range(HKT):
            gemm_k(g, e_own, g, start=(g == 0), stop=False)
        for g in range(HKT):
            gemm_k(HKT + g, e_sib, g, start=False, stop=(g == HKT - 1))

        for mb in range(MBT):
            o_t = outp.tile([P, OH], f32, tag="o")
            nc.vector.tensor_copy(o_t, ps_tiles[mb][:])
            nc.sync.dma_start(out=out_d[mb * P : (mb + 1) * P, :], in_=o_t)

    nc.compile()
    return nc


def kernel(x, weight, uniform, T):
    global _PROGRAM, LAST_RESULT
    import ml_dtypes
    from concourse.bass_utils import run_bass_kernel_spmd

    if _PROGRAM is None:
        _PROGRAM = _build_program()
    nc = _PROGRAM

    x = np.asarray(x, dtype=np.float32)
    weight = np.asarray(weight, dtype=np.float32)
    uniform = np.asarray(uniform, dtype=np.float32)
    T = np.ascontiguousarray(np.asarray(T, dtype=np.float32)).reshape([1])

    xt = np.ascontiguousarray(x.T).astype(ml_dtypes.bfloat16)  # [IN, B] bf16
    in_maps = []
    for c in range(NCORES):
        p, q = c // GO, c % GO
        rows_own = slice(q * RS, (q + 1) * RS)
        rows_sib = slice((1 - q) * RS, (2 - q) * RS)
        cols_perm = np.r_[q * OH : (q + 1) * OH, (1 - q) * OH : (2 - q) * OH]
        xt_perm = np.concatenate([xt[rows_own], xt[rows_sib]], axis=0)
        xt_slice = xt_perm[:, p * BS : (p + 1) * BS]
        # pretile to [P, KT*BS]: row g*128+p -> [p, g*BS : (g+1)*BS]
        xt_tiled = np.ascontiguousarray(
            xt_slice.reshape(KT, P, BS).transpose(1, 0, 2).reshape(P, KT * BS)
        )
        in_maps.append(
            {
                "xt": xt_tiled,
                "wh": np.ascontiguousarray(weight[rows_own][:, cols_perm]),
                "uh": np.ascontiguousarray(uniform[rows_own][:, cols_perm]),
                "tt": T,
            }
        )

    res = run_bass_kernel_spmd(nc, in_maps, core_ids=list(range(NCORES)))
    LAST_RESULT = res

    out = np.empty((B, OUT), dtype=np.float32)
    for c in range(NCORES):
        p, q = c // GO, c % GO
        out[p * BS : (p + 1) * BS, q * OH : (q + 1) * OH] = res.results[c]["out"]
    return out
